# revision 8
# baseline (speedup 1.0000x reference)
import numpy as np
import ml_dtypes

import concourse.bass as bass
import concourse.bacc as bacc
import concourse.tile as tile
from concourse import mybir
from concourse import bass_utils

# Problem dims (hardcoded per contract)
B, S, I, H, O = 64, 2048, 256, 512, 2
NCORES = 8
BL = B // NCORES   # 8 batch rows per core
TB = 64            # timesteps per PSUM block (one [128,2048] f32 = 4 banks)
NBLK = S // TB     # 32 blocks
XSUP = 256         # timesteps per x DMA super-block
F32 = mybir.dt.float32
BF16 = mybir.dt.bfloat16
F8 = mybir.dt.float8e4

_cache = {}


def _build():
    nc = bacc.Bacc("TRN2", target_bir_lowering=False, debug=False,
                   enable_asserts=False)

    xt = nc.dram_tensor("xt", [I, S * BL], BF16, kind="ExternalInput").ap()
    u = nc.dram_tensor("U_w", [H, H], F8, kind="ExternalInput").ap()
    w = nc.dram_tensor("W_w", [I, H], BF16, kind="ExternalInput").ap()
    wb = nc.dram_tensor("wbias", [1, H], BF16, kind="ExternalInput").ap()
    v = nc.dram_tensor("V_w", [128, 4 * O], BF16, kind="ExternalInput").ap()
    vb = nc.dram_tensor("vbias", [1, O], BF16, kind="ExternalInput").ap()
    out = nc.dram_tensor("out", [BL, O], F32, kind="ExternalOutput").ap()

    Tanh = mybir.ActivationFunctionType.Tanh
    Sigmoid = mybir.ActivationFunctionType.Sigmoid

    from contextlib import ExitStack
    with tile.TileContext(nc) as tc, ExitStack() as ctx:
        cpool = ctx.enter_context(tc.tile_pool(name="const", bufs=1))
        xpool = ctx.enter_context(tc.tile_pool(name="xsup", bufs=2))
        hpool = ctx.enter_context(tc.tile_pool(name="h", bufs=2))
        wxpool = ctx.enter_context(tc.tile_pool(name="wx", bufs=2, space="PSUM"))

        # ---- resident constants ----
        u_sb = [cpool.tile([128, H], F8, tag=f"u{k}", name=f"u{k}")
                for k in range(4)]
        for k in range(4):
            nc.sync.dma_start(u_sb[k][:], u[128 * k:128 * (k + 1), :])
        w_sb = [cpool.tile([128, H], BF16, tag=f"w{k}", name=f"w{k}")
                for k in range(2)]
        for k in range(2):
            nc.sync.dma_start(w_sb[k][:], w[128 * k:128 * (k + 1), :])
        wb_sb = cpool.tile([1, H], BF16, tag="wb", name="wb")
        nc.sync.dma_start(wb_sb[:], wb[:, :])
        v_sb = cpool.tile([128, 4 * O], BF16, tag="v", name="v")
        nc.sync.dma_start(v_sb[:], v[:, :])
        vb_sb = cpool.tile([1, O], BF16, tag="vb", name="vb")
        nc.sync.dma_start(vb_sb[:], vb[:, :])
        ones_sb = cpool.tile([1, H], BF16, tag="ones", name="ones")
        nc.gpsimd.memset(ones_sb[:], 1.0)

        BPS = XSUP // TB          # blocks per x super-block
        NSUP = NBLK // BPS
        xtiles = {}

        def dma_super(sup):
            xa = xpool.tile([128, BL * XSUP], BF16, tag="xa", name="xa")
            xb = xpool.tile([128, BL * XSUP], BF16, tag="xb", name="xb")
            c0 = sup * BL * XSUP
            nc.sync.dma_start(xa[:], xt[0:128, c0:c0 + BL * XSUP])
            nc.sync.dma_start(xb[:], xt[128:256, c0:c0 + BL * XSUP])
            xtiles[sup] = (xa, xb)

        def prefill_mm(ps_dst, blk, j):
            # j-th of 12 prefill matmuls for block blk into its psum tile
            m, which = divmod(j, 3)
            xa, xb = xtiles[blk // BPS]
            xoff = (blk % BPS) * BL * TB
            dst = ps_dst[:, 512 * m:512 * (m + 1)]
            if which == 0:
                nc.tensor.matmul(dst, w_sb[0][:, 128 * m:128 * (m + 1)],
                                 xa[:, xoff:xoff + BL * TB],
                                 start=True, stop=False, skip_group_check=True)
            elif which == 1:
                nc.tensor.matmul(dst, w_sb[1][:, 128 * m:128 * (m + 1)],
                                 xb[:, xoff:xoff + BL * TB],
                                 start=False, stop=False, skip_group_check=True)
            else:
                nc.tensor.matmul(dst, wb_sb[:, 128 * m:128 * (m + 1)],
                                 ones_sb[:, 0:BL * TB],
                                 start=False, stop=False, skip_group_check=True)

        dma_super(0)
        dma_super(1)

        # prologue: prefill block 0 entirely
        ps_cur = wxpool.tile([128, 4 * TB * BL], F32, tag="wx", name="wx")
        for j in range(12):
            prefill_mm(ps_cur, 0, j)

        hT01 = hT23 = None   # [128, 16] bf16: col 8*kk + b
        ps_next = None

        for blk in range(NBLK):
            if blk + 1 < NBLK:
                ps_next = wxpool.tile([128, 4 * TB * BL], F32, tag="wx", name="wx")
                if (blk + 1) % BPS == 0:
                    sup = (blk + 1) // BPS
                    if sup + 1 < NSUP:
                        dma_super(sup + 1)
            for r in range(TB):
                t = TB * blk + r
                # interleave next block's prefill into this block's steps
                if blk + 1 < NBLK and r % 5 == 4 and (r - 4) // 5 < 12:
                    prefill_mm(ps_next, blk + 1, (r - 4) // 5)

                last = (r == TB - 1)
                if t > 0:
                    # P1 {m01}x{k01}  P2 {m01}x{k23}  P3 {m23}x{k01}  P4 {m23}x{k23}
                    for (ms, ks, hp) in (((0, 1), (0, 1), hT01),
                                         ((0, 1), (2, 3), hT23),
                                         ((2, 3), (0, 1), hT01),
                                         ((2, 3), (2, 3), hT23)):
                        for m in ms:
                            o0 = 512 * m + BL * r
                            for k in ks:
                                kk = k % 2
                                nc.tensor.matmul(
                                    ps_cur[:, o0:o0 + BL],
                                    u_sb[k][:, 128 * m:128 * (m + 1)],
                                    hp[:, BL * kk:BL * (kk + 1)],
                                    start=False,
                                    stop=(last and k in (1, 3) and k == ks[1]
                                          and m == ms[1]),
                                    skip_group_check=True)
                        if ms == (0, 1) and ks == (2, 3):
                            # psT m0/m1 complete -> ACT1 while P3/P4 run
                            hT01 = _act_half(nc, tc, hpool, ps_cur, r, 0, Tanh)
                    hT23 = _act_half(nc, tc, hpool, ps_cur, r, 2, Tanh)
                else:
                    hT01 = _act_half(nc, tc, hpool, ps_cur, r, 0, Tanh)
                    hT23 = _act_half(nc, tc, hpool, ps_cur, r, 2, Tanh)
            ps_cur = ps_next

        # ---- output head: o = sigmoid(h @ V + Vb) ----
        pso = wxpool.tile([BL, O], F32, tag="wx", name="pso")
        nc.tensor.matmul(pso[:], ones_sb[:, 0:BL], vb_sb[:],
                         start=True, stop=False)
        for k in range(4):
            hp = hT01 if k < 2 else hT23
            kk = k % 2
            nc.tensor.matmul(pso[:], hp[:, BL * kk:BL * (kk + 1)],
                             v_sb[:, O * k:O * (k + 1)],
                             start=False, stop=(k == 3))
        o_sb = hpool.tile([BL, O], F32, tag="osb", name="osb")
        nc.scalar.activation(o_sb[:], pso[:], Sigmoid)
        nc.sync.dma_start(out[:, :], o_sb[:])

    nc.compile()
    return nc


def _act_half(nc, tc, hpool, ps, r, m0, func):
    # tanh over banks m0,m0+1 slice r -> new hT half tile [128, 16] bf16
    ht = hpool.tile([128, 2 * BL], BF16, tag=f"hT{m0}", name=f"hT{m0}")
    view = ps[:].rearrange("p (m n) -> p m n", m=4)
    src = view[:, m0:m0 + 2, BL * r:BL * (r + 1)]
    nc.scalar.activation(ht[:].rearrange("p (m n) -> p m n", m=2), src, func)
    return ht


def kernel(x, W_w, W_b, U_w, U_b, V_w, V_b):
    if "nc" not in _cache:
        _cache["nc"] = _build()
    nc = _cache["nc"]

    bf = ml_dtypes.bfloat16
    v_pack = np.zeros((128, 4 * O), dtype=bf)
    for k in range(4):
        v_pack[:, O * k:O * (k + 1)] = np.asarray(V_w, np.float32)[128 * k:128 * (k + 1), :].astype(bf)
    shared = {
        "U_w": np.ascontiguousarray(np.asarray(U_w, np.float32).astype(ml_dtypes.float8_e4m3)),
        "W_w": np.ascontiguousarray(np.asarray(W_w, np.float32).astype(bf)),
        "wbias": np.ascontiguousarray(
            (np.asarray(W_b, np.float32) + np.asarray(U_b, np.float32))
            .reshape(1, H).astype(bf)),
        "V_w": v_pack,
        "vbias": np.ascontiguousarray(np.asarray(V_b, np.float32).reshape(1, O).astype(bf)),
    }
    x = np.asarray(x, np.float32)
    in_maps = []
    for c in range(NCORES):
        xc = x[c * BL:(c + 1) * BL]                       # [BL, S, I]
        xtc = np.ascontiguousarray(
            xc.transpose(2, 1, 0).reshape(I, S * BL).astype(bf))
        in_maps.append(dict(shared, xt=xtc))

    _cache["in_maps"] = in_maps
    res = bass_utils.run_bass_kernel_spmd(nc, in_maps, core_ids=list(range(NCORES)))
    _cache["last_result"] = res
    return np.concatenate([r["out"] for r in res.results], axis=0)


# revision 9
# speedup vs baseline: 1.4854x; 1.4854x over previous
import numpy as np
import ml_dtypes

import concourse.bass as bass
import concourse.bacc as bacc
import concourse.tile as tile
from concourse import mybir
from concourse import bass_utils

# Problem dims (hardcoded per contract)
B, S, I, H, O = 64, 2048, 256, 512, 2
NCORES = 8
BL = B // NCORES   # 8 batch rows per core
TB = 64            # timesteps per PSUM block (one [128,2048] f32 = 4 banks)
NBLK = S // TB     # 32 blocks
XSUP = 256         # timesteps per x DMA super-block
F32 = mybir.dt.float32
BF16 = mybir.dt.bfloat16
F8 = mybir.dt.float8e4

_cache = {}


def _build():
    nc = bacc.Bacc("TRN2", target_bir_lowering=False, debug=False,
                   enable_asserts=False)

    xt = nc.dram_tensor("xt", [I, S * BL], BF16, kind="ExternalInput").ap()
    u = nc.dram_tensor("U_w", [H, H], F8, kind="ExternalInput").ap()
    w = nc.dram_tensor("W_w", [I, H], BF16, kind="ExternalInput").ap()
    wb = nc.dram_tensor("wbias", [1, H], BF16, kind="ExternalInput").ap()
    v = nc.dram_tensor("V_w", [128, 4 * O], BF16, kind="ExternalInput").ap()
    vb = nc.dram_tensor("vbias", [1, O], BF16, kind="ExternalInput").ap()
    out = nc.dram_tensor("out", [BL, O], F32, kind="ExternalOutput").ap()

    Tanh = mybir.ActivationFunctionType.Tanh
    Sigmoid = mybir.ActivationFunctionType.Sigmoid

    from contextlib import ExitStack
    with tile.TileContext(nc) as tc, ExitStack() as ctx:
        cpool = ctx.enter_context(tc.tile_pool(name="const", bufs=1))
        xpool = ctx.enter_context(tc.tile_pool(name="xsup", bufs=2))
        hpool = ctx.enter_context(tc.tile_pool(name="h", bufs=2))
        wxpool = ctx.enter_context(tc.tile_pool(name="wx", bufs=2, space="PSUM"))

        # ---- resident constants ----
        u_sb = [cpool.tile([128, H], F8, tag=f"u{k}", name=f"u{k}")
                for k in range(4)]
        for k in range(4):
            nc.sync.dma_start(u_sb[k][:], u[128 * k:128 * (k + 1), :])
        w_sb = [cpool.tile([128, H], BF16, tag=f"w{k}", name=f"w{k}")
                for k in range(2)]
        for k in range(2):
            nc.sync.dma_start(w_sb[k][:], w[128 * k:128 * (k + 1), :])
        wb_sb = cpool.tile([1, H], BF16, tag="wb", name="wb")
        nc.sync.dma_start(wb_sb[:], wb[:, :])
        v_sb = cpool.tile([128, 4 * O], BF16, tag="v", name="v")
        nc.sync.dma_start(v_sb[:], v[:, :])
        vb_sb = cpool.tile([1, O], BF16, tag="vb", name="vb")
        nc.sync.dma_start(vb_sb[:], vb[:, :])
        ones_sb = cpool.tile([1, H], BF16, tag="ones", name="ones")
        nc.gpsimd.memset(ones_sb[:], 1.0)

        BPS = XSUP // TB          # blocks per x super-block
        NSUP = NBLK // BPS
        xtiles = {}

        def dma_super(sup):
            xa = xpool.tile([128, BL * XSUP], BF16, tag="xa", name="xa")
            xb = xpool.tile([128, BL * XSUP], BF16, tag="xb", name="xb")
            c0 = sup * BL * XSUP
            nc.sync.dma_start(xa[:], xt[0:128, c0:c0 + BL * XSUP])
            nc.sync.dma_start(xb[:], xt[128:256, c0:c0 + BL * XSUP])
            xtiles[sup] = (xa, xb)

        def prefill_mm(ps_dst, blk, j):
            # j-th of 12 prefill matmuls for block blk into its psum tile
            m, which = divmod(j, 3)
            xa, xb = xtiles[blk // BPS]
            xoff = (blk % BPS) * BL * TB
            dst = ps_dst[:, 512 * m:512 * (m + 1)]
            if which == 0:
                nc.tensor.matmul(dst, w_sb[0][:, 128 * m:128 * (m + 1)],
                                 xa[:, xoff:xoff + BL * TB],
                                 start=True, stop=False, skip_group_check=True)
            elif which == 1:
                nc.tensor.matmul(dst, w_sb[1][:, 128 * m:128 * (m + 1)],
                                 xb[:, xoff:xoff + BL * TB],
                                 start=False, stop=False, skip_group_check=True)
            else:
                nc.tensor.matmul(dst, wb_sb[:, 128 * m:128 * (m + 1)],
                                 ones_sb[:, 0:BL * TB],
                                 start=False, stop=False, skip_group_check=True)

        dma_super(0)
        dma_super(1)

        # prologue: prefill block 0 entirely
        ps_cur = wxpool.tile([128, 4 * TB * BL], F32, tag="wx", name="wx")
        for j in range(12):
            prefill_mm(ps_cur, 0, j)

        hT = None   # [128, 32] bf16: col 8*k + b
        ps_next = None

        for blk in range(NBLK):
            if blk + 1 < NBLK:
                ps_next = wxpool.tile([128, 4 * TB * BL], F32, tag="wx", name="wx")
                if (blk + 1) % BPS == 0:
                    sup = (blk + 1) // BPS
                    if sup + 1 < NSUP:
                        dma_super(sup + 1)
            for r in range(TB):
                t = TB * blk + r
                # interleave next block's prefill into this block's steps
                if blk + 1 < NBLK and r % 5 == 4 and (r - 4) // 5 < 12:
                    prefill_mm(ps_next, blk + 1, (r - 4) // 5)

                last = (r == TB - 1)
                if t > 0:
                    for m in range(4):
                        o0 = 512 * m + BL * r
                        for k in range(4):
                            nc.tensor.matmul(
                                ps_cur[:, o0:o0 + BL],
                                u_sb[k][:, 128 * m:128 * (m + 1)],
                                hT[:, BL * k:BL * (k + 1)],
                                start=False,
                                stop=(last and k == 3),
                                skip_group_check=True)
                hT = _act_full(nc, tc, hpool, ps_cur, r, Tanh)
            ps_cur = ps_next

        # ---- output head: o = sigmoid(h @ V + Vb) ----
        pso = wxpool.tile([BL, O], F32, tag="wx", name="pso")
        nc.tensor.matmul(pso[:], ones_sb[:, 0:BL], vb_sb[:],
                         start=True, stop=False)
        for k in range(4):
            nc.tensor.matmul(pso[:], hT[:, BL * k:BL * (k + 1)],
                             v_sb[:, O * k:O * (k + 1)],
                             start=False, stop=(k == 3))
        o_sb = hpool.tile([BL, O], F32, tag="osb", name="osb")
        nc.scalar.activation(o_sb[:], pso[:], Sigmoid)
        nc.sync.dma_start(out[:, :], o_sb[:])

    nc.compile()
    return nc


def _act_full(nc, tc, hpool, ps, r, func):
    # tanh over all 4 banks slice r -> new hT tile [128, 32] bf16
    ht = hpool.tile([128, 4 * BL], BF16, tag="hT", name="hT")
    view = ps[:].rearrange("p (m n) -> p m n", m=4)
    src = view[:, :, BL * r:BL * (r + 1)]
    nc.scalar.activation(ht[:].rearrange("p (m n) -> p m n", m=4), src, func)
    return ht


def kernel(x, W_w, W_b, U_w, U_b, V_w, V_b):
    if "nc" not in _cache:
        _cache["nc"] = _build()
    nc = _cache["nc"]

    bf = ml_dtypes.bfloat16
    v_pack = np.zeros((128, 4 * O), dtype=bf)
    for k in range(4):
        v_pack[:, O * k:O * (k + 1)] = np.asarray(V_w, np.float32)[128 * k:128 * (k + 1), :].astype(bf)
    shared = {
        "U_w": np.ascontiguousarray(np.asarray(U_w, np.float32).astype(ml_dtypes.float8_e4m3)),
        "W_w": np.ascontiguousarray(np.asarray(W_w, np.float32).astype(bf)),
        "wbias": np.ascontiguousarray(
            (np.asarray(W_b, np.float32) + np.asarray(U_b, np.float32))
            .reshape(1, H).astype(bf)),
        "V_w": v_pack,
        "vbias": np.ascontiguousarray(np.asarray(V_b, np.float32).reshape(1, O).astype(bf)),
    }
    x = np.asarray(x, np.float32)
    in_maps = []
    for c in range(NCORES):
        xc = x[c * BL:(c + 1) * BL]                       # [BL, S, I]
        xtc = np.ascontiguousarray(
            xc.transpose(2, 1, 0).reshape(I, S * BL).astype(bf))
        in_maps.append(dict(shared, xt=xtc))

    _cache["in_maps"] = in_maps
    res = bass_utils.run_bass_kernel_spmd(nc, in_maps, core_ids=list(range(NCORES)))
    _cache["last_result"] = res
    return np.concatenate([r["out"] for r in res.results], axis=0)
